# revision 1
# baseline (speedup 1.0000x reference)
"""GuidedFusion attention kernel for 8x Trainium2 NeuronCores.

Reference computation (per batch b):
    q[l, j] = sum_c low[c, l]  * Wq[j, c] + bq[j]          # [Nl, qd]
    k[j, n] = sum_c high[c, n] * Wk[j, c] + bk[j]          # [qd, Nh]
    E[l, n] = sum_j q[l, j] * k[j, n]                      # [Nl, Nh]
    A       = softmax(E, axis=n)
    O[c, l] = sum_n high[c, n] * A[l, n]                   # [C, Nl]
    out     = gamma * O + low

Strategy: data-parallel over batch B=8 across the 8 cores (one batch each,
no collectives).  Within a core:
  - everything on the tensor engine runs in bf16 with f32 PSUM accumulation
  - the energy is computed *transposed* (E^T[n, l]) so softmax's reduction
    over n lands on the PSUM partition dim, where a ones-matmul computes the
    denominators (already broadcast to 128 partitions) while the value
    matmul consumes the un-normalised exp(E^T) tiles directly -- no on-chip
    transposes of the big attention matrix at all.
  - exp() needs no max-subtraction: energies here are ~N(0, 0.67), |E| < 10
    for these input scales, far inside f32/bf16 exp range, and the softmax
    ratio is mathematically unchanged.
  - gamma is folded into the value matrix host-side; normalisation (1/sum)
    and the residual add are fused into the PSUM->SBUF drain of the output.

Host-side staging per core: f32 residual copy of low, bf16 copies of the
matmul operands, transposed weights/values (free on host, avoids on-chip
transposes).  All shapes are hardcoded for the graded problem size.
"""

import numpy as np
import ml_dtypes

B, C = 8, 256
HL, WL, HH, WH = 64, 64, 32, 32
QD = 64
NL, NH = HL * WL, HH * WH  # 4096, 1024
NCORES = 8
LBLK = 512                 # l-columns per block (one PSUM bank of f32)
NLB = NL // LBLK           # 8 l-blocks
NHC = NH // 128            # 8 key-position chunks

_NC_CACHE = {}


def _build_nc():
    from contextlib import ExitStack

    import concourse.bacc as bacc
    import concourse.mybir as mybir
    import concourse.tile as tile

    f32 = mybir.dt.float32
    bf16 = mybir.dt.bfloat16
    AF = mybir.ActivationFunctionType

    nc = bacc.Bacc(
        "TRN2", target_bir_lowering=False, debug=False, num_devices=NCORES
    )

    lowf = nc.dram_tensor("lowf", [C, NL], f32, kind="ExternalInput")
    lowb = nc.dram_tensor("lowb", [C, NL], bf16, kind="ExternalInput")
    highb = nc.dram_tensor("highb", [C, NH], bf16, kind="ExternalInput")
    vtb = nc.dram_tensor("vtb", [NH, C], bf16, kind="ExternalInput")
    wqt = nc.dram_tensor("wqt", [C, QD], bf16, kind="ExternalInput")
    wkt = nc.dram_tensor("wkt", [C, QD], bf16, kind="ExternalInput")
    bqv = nc.dram_tensor("bqv", [QD, 1], f32, kind="ExternalInput")
    bkv = nc.dram_tensor("bkv", [QD, 1], f32, kind="ExternalInput")
    outd = nc.dram_tensor("out", [C, NL], f32, kind="ExternalOutput")

    with tile.TileContext(nc) as tc, ExitStack() as ctx:
        const = ctx.enter_context(tc.tile_pool(name="const", bufs=1))
        work = ctx.enter_context(tc.tile_pool(name="work", bufs=8))
        outp = ctx.enter_context(tc.tile_pool(name="outp", bufs=4))
        # PSUM banks: psw(e/proj) 3 + o0 2 + o1 2 + s 1 = 8 (the full PSUM)
        ps_w = ctx.enter_context(tc.tile_pool(name="ps_w", bufs=3, space="PSUM"))
        ps_o = ctx.enter_context(tc.tile_pool(name="ps_o", bufs=2, space="PSUM"))
        ps_s = ctx.enter_context(tc.tile_pool(name="ps_s", bufs=1, space="PSUM"))

        # DMA order = consumption order: k-proj deps first, then q/value
        # deps, then the low_level stream (512-col slices so consumers start
        # as soon as their slice lands, not after a full 2 MiB chunk)
        wkt_sb = const.tile([128, 2, QD], bf16, tag="wkt")
        nc.gpsimd.dma_start(out=wkt_sb, in_=wkt[:].rearrange("(c p) m -> p c m", p=128))
        bk_sb = const.tile([QD, 1], f32, tag="bk")
        nc.gpsimd.dma_start(out=bk_sb, in_=bkv[:])
        wqt_sb = const.tile([128, 2, QD], bf16, tag="wqt")
        nc.gpsimd.dma_start(out=wqt_sb, in_=wqt[:].rearrange("(c p) m -> p c m", p=128))
        bq_sb = const.tile([QD, 1], f32, tag="bq")
        nc.gpsimd.dma_start(out=bq_sb, in_=bqv[:])
        # half-chunk tiles so the first k-proj matmul starts after 0.25 MiB
        highb_sb = [
            [const.tile([128, 512], bf16, tag=f"highb{i}_{n}", name=f"highb{i}_{n}")
             for n in range(2)]
            for i in range(2)
        ]
        for n in range(2):
            for i in range(2):
                nc.sync.dma_start(
                    out=highb_sb[i][n],
                    in_=highb[i * 128:(i + 1) * 128, n * 512:(n + 1) * 512],
                )
        ones_sb = const.tile([128, 128], bf16, tag="ones")
        nc.vector.memset(ones_sb, 1.0)
        # touch ACT immediately so its function-table load (~1.3us) runs
        # during the DMA warmup instead of on the first exp's critical path
        warm_sb = const.tile([1, 1], f32, tag="warm")
        nc.vector.memset(warm_sb, 0.0)
        nc.scalar.activation(out=warm_sb, in_=warm_sb, func=AF.Exp)
        lowb_sb = [
            [const.tile([128, 512], bf16, tag=f"lowb{i}_{n}", name=f"lowb{i}_{n}")
             for n in range(NLB)]
            for i in range(2)
        ]
        vtb_sb = const.tile([128, NHC, C], bf16, tag="vtb")

        def dma_lowb(n):
            for i in range(2):
                nc.sync.dma_start(
                    out=lowb_sb[i][n],
                    in_=lowb[i * 128:(i + 1) * 128, n * 512:(n + 1) * 512],
                )

        nc.scalar.dma_start(out=vtb_sb, in_=vtb[:].rearrange("(n p) c -> p n c", p=128))
        for n in range(NLB):
            dma_lowb(n)
        lowf_sb = [
            [const.tile([128, 512], f32, tag=f"lowf{i}_{n}", name=f"lowf{i}_{n}")
             for n in range(NLB)]
            for i in range(2)
        ]
        for n in range(NLB):
            for i in range(2):
                nc.sync.dma_start(
                    out=lowf_sb[i][n],
                    in_=lowf[i * 128:(i + 1) * 128, n * 512:(n + 1) * 512],
                )

        # q lives as one tile per 512-slice so the per-slice projections can
        # interleave with the attention stream without false tile deps
        q_tiles = [const.tile([QD, 512], bf16, tag=f"q{n}", name=f"q{n}")
                   for n in range(NLB)]
        k_sb = const.tile([QD, NH], bf16, tag="k")

        # k projection: k[j, n] = sum_c WkT[c, j] * high[c, n] + bk
        for n in range(NH // 512):
            cols = slice(n * 512, (n + 1) * 512)
            kp = ps_w.tile([QD, 512], f32, tag="psw")
            for cc in range(2):
                nc.tensor.matmul(
                    kp, wkt_sb[:, cc, :], highb_sb[cc][n],
                    start=(cc == 0), stop=(cc == 1),
                )
            nc.vector.tensor_scalar_add(k_sb[:, cols], kp, bk_sb)

        # q projection for one 512-slice: q[j, l] = sum_c WqT[c,j]*low[c,l]+bq
        def emit_qproj(n):
            qp = ps_w.tile([QD, 512], f32, tag="psw")
            for cc in range(2):
                nc.tensor.matmul(
                    qp, wqt_sb[:, cc, :], lowb_sb[cc][n],
                    start=(cc == 0), stop=(cc == 1),
                )
            nc.vector.tensor_scalar_add(q_tiles[n], qp, bq_sb)

        # attention: one flat stream of (l-block, h-chunk) tiles, with the
        # energy matmul software-pipelined DEPTH slots ahead of the value
        # matmuls so the ACT exp latency never lands on PE's critical path.
        # exp chunks are pre-summed pairs->quads on DVE so the softmax-
        # denominator ones-matmul runs at quarter rate (PE is the bottleneck).
        DEPTH = 3
        chunks = [(lb, hc) for lb in range(NLB) for hc in range(NHC)]
        o_ps = {}
        s_ps = {}
        a_tiles = {}
        pair_tiles = {}

        def emit_energy(i):
            lb, hc = chunks[i]
            if hc == 0 and lb + 2 < NLB:
                emit_qproj(lb + 2)  # keep q two blocks ahead of consumption
            e_ps = ps_w.tile([128, LBLK], f32, tag="psw")
            nc.tensor.matmul(
                e_ps, k_sb[:, hc * 128:(hc + 1) * 128], q_tiles[lb],
                start=True, stop=True,
            )
            a_sb = work.tile([128, LBLK], bf16, tag="aexp")
            nc.scalar.activation(out=a_sb, in_=e_ps, func=AF.Exp)
            a_tiles[i] = a_sb

        def emit_value(i):
            lb, hc = chunks[i]
            a_sb = a_tiles[i]
            first, last = hc == 0, hc == NHC - 1
            if first:
                o_ps[lb] = [
                    ps_o.tile([128, LBLK], f32, tag=f"o{j}", name=f"o{j}")
                    for j in range(2)
                ]
                s_ps[lb] = ps_s.tile([128, LBLK], f32, tag="s", name="s")
            nc.tensor.matmul(
                o_ps[lb][0], vtb_sb[:, hc, 0:128], a_sb, start=first, stop=last
            )
            nc.tensor.matmul(
                o_ps[lb][1], vtb_sb[:, hc, 128:256], a_sb, start=first, stop=last
            )
            if hc % 2 == 1:
                pair = work.tile([128, LBLK], bf16, tag="apair")
                nc.vector.tensor_add(pair, a_tiles.pop(i - 1), a_tiles.pop(i))
                pair_tiles[hc // 2] = pair
            if hc % 4 == 3:
                quad = work.tile([128, LBLK], bf16, tag="aquad")
                nc.vector.tensor_add(
                    quad, pair_tiles.pop(hc // 2 - 1), pair_tiles.pop(hc // 2)
                )
                nc.tensor.matmul(
                    s_ps[lb], ones_sb, quad, start=(hc == 3), stop=last
                )
            if last:
                lcols = slice(lb * LBLK, (lb + 1) * LBLK)
                rs = outp.tile([128, LBLK], f32, tag="rs")
                nc.vector.reciprocal(out=rs, in_=s_ps.pop(lb))
                ob = o_ps.pop(lb)
                for cc in range(2):
                    rows = slice(cc * 128, (cc + 1) * 128)
                    t = outp.tile([128, LBLK], f32, tag=f"t{cc}")
                    nc.vector.tensor_mul(t, ob[cc], rs)
                    add_eng = nc.vector if lb == NLB - 1 else nc.gpsimd
                    add_eng.tensor_add(t, t, lowf_sb[cc][lb])
                    nc.sync.dma_start(out=outd[rows, lcols], in_=t)

        emit_qproj(0)
        if NLB > 1:
            emit_qproj(1)
        for i in range(len(chunks) + DEPTH):
            if i < len(chunks):
                emit_energy(i)
            if i >= DEPTH:
                emit_value(i - DEPTH)

    nc.compile()
    return nc


def _get_nc():
    if "nc" not in _NC_CACHE:
        _NC_CACHE["nc"] = _build_nc()
    return _NC_CACHE["nc"]


def kernel(low_level, high_level, Wq, bq, Wk, bk, gamma, **_unused):
    from concourse.bass_utils import run_bass_kernel_spmd

    bf16 = ml_dtypes.bfloat16
    low = np.ascontiguousarray(np.asarray(low_level, np.float32)).reshape(B, C, NL)
    high = np.ascontiguousarray(np.asarray(high_level, np.float32)).reshape(B, C, NH)
    g = float(np.asarray(gamma, np.float32).reshape(-1)[0])
    wqt_h = np.ascontiguousarray(np.asarray(Wq, np.float32).T).astype(bf16)
    wkt_h = np.ascontiguousarray(np.asarray(Wk, np.float32).T).astype(bf16)
    bqv_h = np.asarray(bq, np.float32).reshape(QD, 1).copy()
    bkv_h = np.asarray(bk, np.float32).reshape(QD, 1).copy()

    in_maps = []
    for b in range(B):
        in_maps.append(
            dict(
                lowf=low[b],
                lowb=low[b].astype(bf16),
                highb=high[b].astype(bf16),
                vtb=np.ascontiguousarray((g * high[b]).T).astype(bf16),
                wqt=wqt_h,
                wkt=wkt_h,
                bqv=bqv_h,
                bkv=bkv_h,
            )
        )

    nc = _get_nc()
    res = run_bass_kernel_spmd(nc, in_maps, core_ids=list(range(NCORES)))
    out = np.stack([res.results[b]["out"] for b in range(B)], axis=0)
    return out.reshape(B, C, HL, WL).astype(np.float32, copy=False)



# revision 2
# speedup vs baseline: 1.0058x; 1.0058x over previous
"""GuidedFusion attention kernel for 8x Trainium2 NeuronCores.

Reference computation (per batch b):
    q[l, j] = sum_c low[c, l]  * Wq[j, c] + bq[j]          # [Nl, qd]
    k[j, n] = sum_c high[c, n] * Wk[j, c] + bk[j]          # [qd, Nh]
    E[l, n] = sum_j q[l, j] * k[j, n]
    A       = softmax(E, axis=n)
    O[c, l] = sum_n high[c, n] * A[l, n]
    out     = gamma * O + low

Strategy: data-parallel over batch B=8 across the 8 cores (one batch each,
no collectives).  Within a core, all heavy matmuls run in fp8 DoubleRow
mode (two contraction blocks per instruction at half cost) against
host-packed [K, 2, M] operands, and softmax runs shift-invariant with a
fixed -2 shift instead of a max-subtraction (energies are ~N(0, 0.67),
|E| < 6 for these input scales, so exp stays inside fp8 range).

  - The energy is computed transposed (E^T[n, l]); the qd=64 contraction
    uses a packed stationary with a zeroed second block plus a stride-0
    duplicated moving operand, so it still gets DoubleRow pricing.
  - The output is computed transposed (O^T[l, c]) so the softmax
    normalizer is a per-partition scalar: denominators come from
    free-size-1 matmuls against a ones vector (nearly free on PE), and
    normalize + residual-add fuse into one scalar_tensor_tensor op.
  - exp splits across three engines: ACT runs real Exp (fp8e5 out);
    DVE/GPSIMD run a fast-exp bit trick (affine f32 -> int8, bitcast to
    fp8e5).  Both paths feed the same numerator and denominator, so the
    softmax stays normalized despite fp8-grade weights.
  - gamma folds into the value matrix host-side; the residual uses a bf16
    low^T copy; the final transpose back to [C, Nl] and the f32 cast
    happen host-side on the gathered output.

Weights are pre-scaled by 64 (and q/k by 4) host-side to dodge fp8
subnormals; the descale folds into existing per-partition scale slots.
All shapes are hardcoded for the graded problem size.
"""

import numpy as np
import ml_dtypes

B, C = 8, 256
HL, WL, HH, WH = 64, 64, 32, 32
QD = 64
NL, NH = HL * WL, HH * WH  # 4096, 1024
NCORES = 8
LBLK = 512                 # l-columns per block
NLB = NL // LBLK           # 8 l-blocks
NT = 4                     # DoubleRow chunk pairs over Nh (4 x 256)
NLC = 4                    # 128-row l-chunks per l-block

_NC_CACHE = {}

# fast-exp: e5m2 bits i approximate exp(x - 2) via i = x*(4*log2 e) + bias;
# energies arrive pre-scaled by 16 (q and k each carry a 4x).
FEXP_MUL = 5.7708 / 16.0
FEXP_ADD = 59.78 - 2.0 * 5.7708


def _exp_engine(lb, t):
    """Split the 32 exp pairs 18/14 ACT/DVE, spread evenly per l-block; the
    last l-block's final pair stays on ACT so the wind-down drains own DVE."""
    if lb in (3, 5):
        return "dve" if t == 1 else "act"      # 3A/1D
    return "act" if t % 2 == 0 else "dve"      # 2A/2D


def _build_nc():
    from contextlib import ExitStack

    import concourse.bacc as bacc
    import concourse.mybir as mybir
    import concourse.tile as tile

    f32 = mybir.dt.float32
    bf16 = mybir.dt.bfloat16
    fp8e4 = mybir.dt.float8e4
    fp8e5 = mybir.dt.float8e5
    i8 = mybir.dt.int8
    AF = mybir.ActivationFunctionType
    ALU = mybir.AluOpType
    DR = mybir.MatmulPerfMode.DoubleRow

    nc = bacc.Bacc(
        "TRN2", target_bir_lowering=False, debug=False, num_devices=NCORES
    )

    # head: per-partition [hp chunk0 (2x512) | lp chunk0 (2x512) | wb (2x2x64)]
    head_d = nc.dram_tensor("head", [128, 2304], fp8e4, kind="ExternalInput")
    lp_d = nc.dram_tensor("lp", [128, 2, NL - 512], fp8e4, kind="ExternalInput")
    hp_d = nc.dram_tensor("hp", [128, 2, 512], fp8e4, kind="ExternalInput")
    vtp_d = nc.dram_tensor("vtp", [128, 2, NT, C + 1], fp8e5, kind="ExternalInput")
    ltp_d = nc.dram_tensor("ltp", [128, NL // 128, C], bf16, kind="ExternalInput")
    bb_d = nc.dram_tensor("bb", [QD, 2], f32, kind="ExternalInput")
    out_d = nc.dram_tensor("out", [128, NL // 128, C], bf16, kind="ExternalOutput")

    with tile.TileContext(nc) as tc, ExitStack() as ctx:
        const = ctx.enter_context(tc.tile_pool(name="const", bufs=1))
        apool = ctx.enter_context(tc.tile_pool(name="apool", bufs=12))
        opool = ctx.enter_context(tc.tile_pool(name="opool", bufs=3))
        # PSUM banks: epair 2x2 + qp 1 + ob 3 = 8
        ps_e = ctx.enter_context(tc.tile_pool(name="ps_e", bufs=2, space="PSUM"))
        ps_q = ctx.enter_context(tc.tile_pool(name="ps_q", bufs=1, space="PSUM"))
        ps_o = ctx.enter_context(tc.tile_pool(name="ps_o", bufs=3, space="PSUM"))

        # --- DMAs: the critical head transfers (weights, first high/low
        # chunks) ride the GPSIMD SWDGE queue, bypassing the serial HWDGE
        # descriptor generator; bulk streams follow on the SP queue --------
        head_sb = const.tile([128, 2304], fp8e4, tag="head")
        nc.sync.dma_start(out=head_sb, in_=head_d[:])
        bb_sb = const.tile([QD, 2], f32, tag="bb")
        nc.sync.dma_start(out=bb_sb, in_=bb_d[:])
        hp0_sb = head_sb[:, 0:1024].rearrange("p (a b) -> p a b", a=2)
        lp0_sb = head_sb[:, 1024:2048].rearrange("p (a b) -> p a b", a=2)
        wb_sb = head_sb[:, 2048:2304].rearrange(
            "p (a w j) -> p a w j", a=2, w=2)
        wkp_sb = wb_sb[:, :, 0, :]
        wqp_sb = wb_sb[:, :, 1, :]
        bk4_sb = bb_sb[:, 0:1]
        bq4_sb = bb_sb[:, 1:2]
        hp_sb = const.tile([128, 2, NH], fp8e4, tag="hp")
        lp_sb = const.tile([128, 2, NL], fp8e4, tag="lp")
        nc.sync.dma_start(out=hp_sb[:, :, 512:NH], in_=hp_d[:])
        nc.sync.dma_start(
            out=lp_sb[:, :, 512:2048], in_=lp_d[:, :, 0:1536])
        vtp_sb = const.tile([128, 2, NT, C + 1], fp8e5, tag="vtp")
        nc.sync.dma_start(out=vtp_sb, in_=vtp_d[:])
        nc.sync.dma_start(out=lp_sb[:, :, 2048:NL], in_=lp_d[:, :, 1536:NL - 512])
        ltp_sb = const.tile([128, NL // 128, C], bf16, tag="ltp")
        for h in range(2):
            nc.sync.dma_start(
                out=ltp_sb[:, h * 16:(h + 1) * 16, :],
                in_=ltp_d[:, h * 16:(h + 1) * 16, :],
            )

        # --- constants ----------------------------------------------------
        warm = const.tile([1, 1], f32, tag="warm")
        nc.vector.memset(warm, 0.0)
        nc.scalar.activation(out=warm, in_=warm, func=AF.Exp)
        ebias = const.tile([128, 1], f32, tag="ebias")
        nc.vector.memset(ebias, -2.0)
        escale = const.tile([128, 1], f32, tag="escale")
        nc.vector.memset(escale, 1.0 / 16.0)
        qscale = const.tile([QD, 1], f32, tag="qscale")
        nc.vector.memset(qscale, 1.0 / 16.0)
        # k packed [qd, 2, Nh]: odd contraction block stays zero
        kpk_sb = const.tile([QD, 2, NH], fp8e4, tag="kpk")
        nc.gpsimd.memset(kpk_sb[:, 1, :], 0.0)

        q_tiles = [const.tile([QD, LBLK], fp8e4, tag=f"q{n}", name=f"q{n}")
                   for n in range(NLB)]
        rs_all = const.tile([128, 32], f32, tag="rs")

        # --- k projection: k4 = 4*(Wk high + bk), DoubleRow over packed C.
        # kp uses the (still idle) output banks so the q projection can run
        # in parallel in its own bank.
        def emit_kproj(nb):
            kp = ps_o.tile([QD, 512], f32, tag="ob", name="kp")
            nc.tensor.matmul(
                kp, wkp_sb,
                hp0_sb if nb == 0 else hp_sb[:, :, 512:1024],
                start=True, stop=True, perf_mode=DR,
            )
            halves = (2 if nb == 0 else 1)
            step = 512 // halves
            for h in range(halves):
                nc.vector.tensor_scalar(
                    out=kpk_sb[:, 0, nb * 512 + h * step:
                               nb * 512 + (h + 1) * step],
                    in0=kp[:, h * step:(h + 1) * step],
                    scalar1=1.0 / 16.0, scalar2=bk4_sb,
                    op0=ALU.mult, op1=ALU.add,
                )

        # --- q projection: q4 = 4*(Wq low + bq) ---------------------------
        def emit_qproj(lb):
            qp = ps_q.tile([QD, 512], f32, tag="qp", name="qp")
            nc.tensor.matmul(
                qp, wqp_sb,
                lp0_sb if lb == 0 else lp_sb[:, :, lb * 512:(lb + 1) * 512],
                start=True, stop=True, perf_mode=DR,
            )
            nc.scalar.activation(
                out=q_tiles[lb], in_=qp, func=AF.Identity,
                bias=bq4_sb, scale=qscale,
            )

        # --- main pipeline ------------------------------------------------
        a_pairs = {}
        out_tiles = {}

        def emit_energy_exp(lb, t):
            e_pair = ps_e.tile([128, 2, 512], f32, tag="ep", name="ep")
            q_dup = q_tiles[lb].unsqueeze(1).broadcast_to((QD, 2, LBLK))
            for r in range(2):
                hc = 2 * t + r
                nc.tensor.matmul(
                    e_pair[:, r, :],
                    kpk_sb[:, :, hc * 128:(hc + 1) * 128],
                    q_dup,
                    start=True, stop=True, perf_mode=DR,
                )
            eng = _exp_engine(lb, t)
            if eng == "act":
                a_sb = apool.tile([128, 2, LBLK], fp8e5, tag="ae", name="ae")
                nc.scalar.activation(
                    out=a_sb.rearrange("p a b -> p (a b)"),
                    in_=e_pair.rearrange("p a b -> p (a b)"),
                    func=AF.Exp, bias=ebias, scale=escale,
                )
                a_mm = a_sb
            else:
                a_i8 = apool.tile([128, 2, LBLK], i8, tag="ai", name="ai")
                nc.vector.tensor_scalar(
                    out=a_i8.rearrange("p a b -> p (a b)"),
                    in0=e_pair.rearrange("p a b -> p (a b)"),
                    scalar1=FEXP_MUL, scalar2=FEXP_ADD,
                    op0=ALU.mult, op1=ALU.add,
                )
                a_mm = a_i8.bitcast(fp8e5)
            a_pairs[(lb, t)] = a_mm

        def emit_values_drains(lb, lcs):
            if lcs[0] == 0:
                out_tiles[lb] = opool.tile(
                    [128, NLC, C], bf16, tag="ob", name="ob")
            out_sb = out_tiles[lb]
            for lc in lcs:
                ob = ps_o.tile([128, 512], f32, tag="ob", name="obp")
                a_lo = lc * 128
                for t in range(NT):
                    nc.tensor.matmul(
                        ob[:, 0:C + 1],
                        a_pairs[(lb, t)][:, :, a_lo:a_lo + 128],
                        vtp_sb[:, :, t, :],
                        start=(t == 0), stop=(t == NT - 1),
                        perf_mode=DR,
                    )
                lcg = lb * NLC + lc
                # the denominator rides the value matmul as column 256;
                # reciprocal, then fused normalize+residual: out = o*rs + low^T
                nc.vector.reciprocal(
                    out=rs_all[:, lcg:lcg + 1], in_=ob[:, C:C + 1])
                if lb % 2 == 1 and lc % 2 == 1 and lb != NLB - 1:
                    # odd l-blocks drain during even-split exp slots: move
                    # half their drains to ACT + GPSIMD to keep DVE level
                    nc.scalar.activation(
                        out=out_sb[:, lc, :], in_=ob[:, 0:C], func=AF.Copy,
                        bias=0.0, scale=rs_all[:, lcg:lcg + 1],
                    )
                    nc.gpsimd.tensor_tensor(
                        out=out_sb[:, lc, :], in0=out_sb[:, lc, :],
                        in1=ltp_sb[:, lcg, :], op=ALU.add,
                    )
                else:
                    nc.vector.scalar_tensor_tensor(
                        out=out_sb[:, lc, :], in0=ob[:, 0:C],
                        scalar=rs_all[:, lcg:lcg + 1],
                        in1=ltp_sb[:, lcg, :],
                        op0=ALU.mult, op1=ALU.add,
                    )
                if lc % 2 == 1:
                    nc.sync.dma_start(
                        out=out_d[:, lb * NLC + lc - 1:lb * NLC + lc + 1, :],
                        in_=out_sb[:, lc - 1:lc + 1, :])
            if lcs[-1] == NLC - 1:
                for t in range(NT):
                    a_pairs.pop((lb, t))
                out_tiles.pop(lb)

        emit_kproj(0)
        emit_qproj(0)
        emit_kproj(1)
        for slot in range(NLB + 1):
            if slot < NLB:
                for t in range(NT):
                    emit_energy_exp(slot, t)
                    if t == 0 and slot + 1 < NLB:
                        emit_qproj(slot + 1)
                    if slot >= 1:
                        if t == 1:
                            emit_values_drains(slot - 1, (0, 1))
                        elif t == 3:
                            emit_values_drains(slot - 1, (2, 3))
            else:
                emit_values_drains(slot - 1, (0, 1))
                emit_values_drains(slot - 1, (2, 3))

    nc.compile()
    return nc


def _get_nc():
    if "nc" not in _NC_CACHE:
        _NC_CACHE["nc"] = _build_nc()
    return _NC_CACHE["nc"]


def _stage_inputs(low_level, high_level, Wq, bq, Wk, bk, gamma):
    e4 = ml_dtypes.float8_e4m3
    e5 = ml_dtypes.float8_e5m2
    bf16 = ml_dtypes.bfloat16

    low = np.ascontiguousarray(np.asarray(low_level, np.float32)).reshape(B, C, NL)
    high = np.ascontiguousarray(np.asarray(high_level, np.float32)).reshape(B, C, NH)
    g = float(np.asarray(gamma, np.float32).reshape(-1)[0])

    wq64 = 64.0 * np.asarray(Wq, np.float32)
    wk64 = 64.0 * np.asarray(Wk, np.float32)
    # wb[k, r, 0, j] = 64*Wk[j, k+128r]; wb[k, r, 1, j] = 64*Wq[...]
    wb_h = np.empty((128, 2, 2, QD), dtype=e4)
    wb_h[:, :, 0, :] = wk64.T.reshape(2, 128, QD).transpose(1, 0, 2).astype(e4)
    wb_h[:, :, 1, :] = wq64.T.reshape(2, 128, QD).transpose(1, 0, 2).astype(e4)
    bb_h = np.stack([
        4.0 * np.asarray(bk, np.float32),
        4.0 * np.asarray(bq, np.float32),
    ], axis=1).astype(np.float32)

    in_maps = []
    for b in range(B):
        lp_full = low[b].reshape(2, 128, NL).transpose(1, 0, 2).astype(e4)
        hp_full = high[b].reshape(2, 128, NH).transpose(1, 0, 2).astype(e4)
        head_h = np.empty((128, 2304), dtype=e4)
        head_h[:, 0:1024] = hp_full[:, :, 0:512].reshape(128, 1024)
        head_h[:, 1024:2048] = lp_full[:, :, 0:512].reshape(128, 1024)
        head_h[:, 2048:2304] = wb_h.reshape(128, 256)
        lp_h = np.ascontiguousarray(lp_full[:, :, 512:NL])
        hp_h = np.ascontiguousarray(hp_full[:, :, 512:NH])
        # vtp[k, r, t, c] = g*high[c, 256 t + 128 r + k]; col C is all-ones
        # so the value matmul also accumulates the softmax denominator
        vtp_h = np.empty((128, 2, NT, C + 1), dtype=e5)
        vtp_h[:, :, :, :C] = (g * high[b]).T.reshape(
            NT, 2, 128, C).transpose(2, 1, 0, 3).astype(e5)
        vtp_h[:, :, :, C] = e5(1.0)
        # ltp[p, i, c] = low[c, 128 i + p]
        ltp_h = np.ascontiguousarray(
            low[b].T.reshape(NL // 128, 128, C).transpose(1, 0, 2)).astype(bf16)
        in_maps.append(
            dict(head=head_h, lp=lp_h, hp=hp_h, vtp=vtp_h, ltp=ltp_h,
                 bb=bb_h)
        )
    return in_maps


def kernel(low_level, high_level, Wq, bq, Wk, bk, gamma, **_unused):
    from concourse.bass_utils import run_bass_kernel_spmd

    in_maps = _stage_inputs(low_level, high_level, Wq, bq, Wk, bk, gamma)
    nc = _get_nc()
    res = run_bass_kernel_spmd(nc, in_maps, core_ids=list(range(NCORES)))
    # out[p, i, c] -> out[b][c, 128 i + p]
    out = np.stack(
        [
            res.results[b]["out"].astype(np.float32).transpose(2, 1, 0).reshape(C, NL)
            for b in range(B)
        ],
        axis=0,
    )
    return out.reshape(B, C, HL, WL)
